# revision 26
# baseline (speedup 1.0000x reference)
"""Hadamard transform kernel for Trainium2 (8 NeuronCores, SPMD data-parallel).

Computes y = (x @ H^T) / sqrt(D), padded with a zero imaginary plane ->
[B, S, D, 2], for x [4, 4096, 1024] fp32 and H the 1024-point Hadamard
matrix (H[i,j] = (-1)^popcount(i&j), symmetric, Kronecker-structured).

Precision/layout choices (all inside kernel(), tolerance is 2e-2):
  - x is rounded to bf16 and pre-transposed per 128-row tile on the host
    during sharding (pure layout + the same rounding the on-chip pipeline
    would apply): halves load traffic and removes all PE transposes.
  - The device writes the real plane in bf16 (host upcasts to fp32 and
    interleaves the zero imaginary plane): halves store traffic.
  Measured end-to-end relative error ~3e-3.

Per-core traffic: 4 MiB in + 4 MiB out + 0.13 MiB weights (~23.7 us of
DMA at the 360 GB/s roofline); the four compute engines are balanced at
~1.2-1.3 us/tile each, right at the per-tile DMA budget.

Math (shard of 2048 rows, 16 row-tiles of 128):
  H_1024 = H_4 (x) H_256  under d = a*256 + b, f = c*256 + e, with
  H_256[e, j*128+b'] = H2[e8, j] * H128[e_lo, b'] (e = e8*128 + e_lo).
  Stage 1 (PE, bf16): per quarter a in 0..4, 2 accumulating matmuls
    z_a += xt[:, (2a+j)*128:...]^T @ W2[:, j*256:(j+1)*256], where
    W2[b', j*256 + e8*128 + e_lo] = H2[e8,j] * H128[e_lo,b'] / 32
    (host-precomputed, exact +-2^-5 entries, bf16).
    All four z quarters accumulate into ONE 2-bank PSUM tile, so PSUM
    staging is a single 1024-wide ACT copy (PSUM f32 -> SBUF bf16).
  Stage 2 (H4 butterfly over a, 256 cols/op, all-SBUF bf16): dist-2
    w0..w3 and dist-1 y0/y1 on DVE (16-bit fast mode), y2/y3 on Pool;
    one full-row store per tile on SP (the shared HWDGE generator costs
    ~625 ns per DMA, so DMA instruction count is kept minimal).
  Startup: all 16 loads queued on SP up front; W rides the ACT queue; a
  burst of dummy matmuls ramps the PE p-state during the first loads.
  The final tile keeps y2/y3 on DVE and splits its store to shorten the
  drain tail.
"""

import numpy as np
from contextlib import ExitStack

import concourse.tile as tile
from concourse import bacc, bass_utils, mybir

N_CORES = 8
B, S, D = 4, 4096, 1024
ROWS = B * S                 # 16384
SHARD = ROWS // N_CORES      # 2048
NT = SHARD // 128            # 16 tiles of 128 rows per core
F32 = mybir.dt.float32
BF16 = mybir.dt.bfloat16

_cache = {}

CFG = {
    "xin_bufs": 16,
    "out_bufs": 8,
    "zs_bufs": 6,
    "w_bufs": 6,
    "z_bufs": 2,
    "warmup": 10,
}


def _build_nc(cfg=None):
    cfg = {**CFG, **(cfg or {})}
    nc = bacc.Bacc("TRN2", target_bir_lowering=False, debug=False)
    # xt: per tile t, xt[t*128+b', g*128+n] = x[t*128+n, g*128+b'] (bf16)
    xt_d = nc.dram_tensor("xt", [SHARD, D], BF16, kind="ExternalInput").ap()
    w_d = nc.dram_tensor("w", [128, 512], BF16, kind="ExternalInput").ap()
    o_d = nc.dram_tensor("out", [SHARD, D], BF16, kind="ExternalOutput").ap()

    with tile.TileContext(nc) as tc, ExitStack() as ctx:
        const_pool = ctx.enter_context(tc.tile_pool(name="const", bufs=1))
        xin_pool = ctx.enter_context(tc.tile_pool(name="xin", bufs=cfg["xin_bufs"]))
        out_pool = ctx.enter_context(tc.tile_pool(name="outp", bufs=cfg["out_bufs"]))
        zs_pool = ctx.enter_context(tc.tile_pool(name="zs", bufs=cfg["zs_bufs"]))
        wb_pool = ctx.enter_context(tc.tile_pool(name="wb", bufs=cfg["w_bufs"]))
        ps_zp = ctx.enter_context(
            tc.tile_pool(name="ps_zp", bufs=cfg["z_bufs"], space="PSUM"))

        # All 16 xt loads queued on SP up front; W rides the ACT queue.
        xt_tiles = []
        for it in range(NT):
            xt_sb = xin_pool.tile([128, D], BF16, tag="xt")
            nc.sync.dma_start(xt_sb[:], xt_d[it * 128:(it + 1) * 128, :])
            xt_tiles.append(xt_sb)

        W_sb = const_pool.tile([128, 512], BF16, tag="W")
        nc.scalar.dma_start(W_sb[:], w_d[:])

        # PE p-state warmup: dummy matmuls on a zeroed tile while the first
        # loads are in flight (reusing the z0 PSUM pool).
        Zb_sb = const_pool.tile([128, 256], BF16, tag="Zb")
        nc.vector.memset(Zb_sb[:], 0.0)
        for _ in range(cfg["warmup"]):
            warm_ps = ps_zp.tile([128, 1024], F32, tag="z")
            nc.tensor.matmul(warm_ps[:, 0:256], lhsT=Zb_sb[:, 0:128], rhs=Zb_sb[:],
                             start=True, stop=True)

        for it in range(NT):
            xt_sb = xt_tiles[it]
            last = it == NT - 1
            # all four z quarters share one 2-bank PSUM tile (four
            # accumulation groups): staging to SBUF is ONE 1024-wide ACT copy
            zp = ps_zp.tile([128, 1024], F32, tag="z")
            zs = zs_pool.tile([128, 1024], BF16, tag="zs")
            for a in range(4):
                zps = zp[:, a * 256:(a + 1) * 256]
                for j in range(2):
                    g = 2 * a + j
                    nc.tensor.matmul(
                        zps,
                        lhsT=xt_sb[:, g * 128:(g + 1) * 128],
                        rhs=W_sb[:, j * 256:(j + 1) * 256],
                        start=(j == 0),
                        stop=(j == 1),
                    )
            nc.scalar.copy(zs[:], zp[:])
            zs01 = zs[:, 0:512]
            zs23 = zs[:, 512:1024]

            # H4 butterfly over the a axis, all-SBUF in bf16 (DVE 4x mode,
            # Pool-eligible). zs01 = [z0|z1], zs23 = [z2|z3].
            w0 = wb_pool.tile([128, 256], BF16, tag="w0")
            w1 = wb_pool.tile([128, 256], BF16, tag="w1")
            w2 = wb_pool.tile([128, 256], BF16, tag="w2")
            w3 = wb_pool.tile([128, 256], BF16, tag="w3")
            nc.vector.tensor_add(w0[:], zs01[:, 0:256], zs23[:, 0:256])
            nc.vector.tensor_sub(w2[:], zs01[:, 0:256], zs23[:, 0:256])
            nc.vector.tensor_add(w1[:], zs01[:, 256:512], zs23[:, 256:512])
            nc.vector.tensor_sub(w3[:], zs01[:, 256:512], zs23[:, 256:512])

            ob = out_pool.tile([128, D], BF16, tag="ob")
            row = o_d[it * 128:(it + 1) * 128, :]
            # dist-1 stage: DVE takes y0/y1 (4x bf16 mode), Pool takes y2/y3.
            # One full store per tile on SP: the shared HWDGE generator costs
            # ~625 ns per DMA, so instruction count matters more than shipping
            # halves early (SWDGE would burn ~1 us of Pool ENGINE per store).
            # The final tile keeps everything on DVE and splits its store so
            # the drain tail is as short as possible.
            nc.vector.tensor_add(ob[:, 0:256], w0[:], w1[:])
            nc.vector.tensor_sub(ob[:, 256:512], w0[:], w1[:])
            if last:
                nc.sync.dma_start(row[:, 0:512], ob[:, 0:512])
                nc.vector.tensor_add(ob[:, 512:768], w2[:], w3[:])
                nc.vector.tensor_sub(ob[:, 768:1024], w2[:], w3[:])
                nc.sync.dma_start(row[:, 512:1024], ob[:, 512:1024])
            else:
                nc.gpsimd.tensor_add(ob[:, 512:768], w2[:], w3[:])
                nc.gpsimd.tensor_sub(ob[:, 768:1024], w2[:], w3[:])
                nc.sync.dma_start(row[:], ob[:])

    nc.compile()
    return nc


def _get_nc():
    if "nc" not in _cache:
        _cache["nc"] = _build_nc()
    return _cache["nc"]


def kernel(x, H, **_ignored):
    import ml_dtypes

    x = np.asarray(x, dtype=np.float32)
    H = np.asarray(H, dtype=np.float32)
    nc = _get_nc()

    # Derive the Kronecker factors from the given H (exact when H has the
    # Hadamard structure); fold in the 1/sqrt(1024) scale.
    R = np.ascontiguousarray(H[:128, :128]) * np.float32(1.0 / 32.0)  # symmetric
    H2s = np.ascontiguousarray(H[:2, :2])  # (-1)^popcount(i&j) signs
    # W2[b', j*256 + e8*128 + e_lo] = H2s[e8, j] * R[b', e_lo]
    W = np.ascontiguousarray(
        np.einsum("ej,bl->bjel", H2s, R).reshape(128, 512)
    ).astype(ml_dtypes.bfloat16)

    # Round x to bf16 (the on-chip pipeline would do the same before the
    # 16-bit matmuls) and pre-transpose per 128-row tile:
    # xt[t, b', g, n] = x[t, n, g, b']
    xb = x.reshape(ROWS // 128, 128, 8, 128).astype(ml_dtypes.bfloat16)
    xt = np.ascontiguousarray(xb.transpose(0, 3, 2, 1)).reshape(ROWS, D)

    in_maps = []
    for c in range(N_CORES):
        in_maps.append({
            "xt": np.ascontiguousarray(xt[c * SHARD:(c + 1) * SHARD]),
            "w": W,
        })

    res = bass_utils.run_bass_kernel_spmd(nc, in_maps, core_ids=list(range(N_CORES)))
    y = np.empty((ROWS, D, 2), dtype=np.float32)
    for c in range(N_CORES):
        y[c * SHARD:(c + 1) * SHARD, :, 0] = res.results[c]["out"].astype(np.float32)
    y[:, :, 1] = 0.0
    return y.reshape(B, S, D, 2)


# revision 28
# speedup vs baseline: 1.0188x; 1.0188x over previous
"""Hadamard transform kernel for Trainium2 (8 NeuronCores, SPMD data-parallel).

Computes y = (x @ H^T) / sqrt(D), padded with a zero imaginary plane ->
[B, S, D, 2], for x [4, 4096, 1024] fp32 and H the 1024-point Hadamard
matrix (H[i,j] = (-1)^popcount(i&j), symmetric, Kronecker-structured).

Precision/layout choices (all inside kernel(), tolerance is 2e-2):
  - x is rounded to bf16 and pre-transposed per 128-row tile on the host
    during sharding (pure layout + the same rounding the on-chip pipeline
    would apply): halves load traffic and removes all PE transposes.
  - The device writes the real plane in bf16 (host upcasts to fp32 and
    interleaves the zero imaginary plane): halves store traffic.
  Measured end-to-end relative error ~3e-3.

Per-core traffic: 4 MiB in + 4 MiB out + 0.13 MiB weights (~23.7 us of
DMA at the 360 GB/s roofline); the four compute engines are balanced at
~1.2-1.3 us/tile each, right at the per-tile DMA budget.

Math (shard of 2048 rows, 16 row-tiles of 128):
  H_1024 = H_4 (x) H_256  under d = a*256 + b, f = c*256 + e, with
  H_256[e, j*128+b'] = H2[e8, j] * H128[e_lo, b'] (e = e8*128 + e_lo).
  Stage 1 (PE, bf16): per quarter a in 0..4, 2 accumulating matmuls
    z_a += xt[:, (2a+j)*128:...]^T @ W2[:, j*256:(j+1)*256], where
    W2[b', j*256 + e8*128 + e_lo] = H2[e8,j] * H128[e_lo,b'] / 32
    (host-precomputed, exact +-2^-5 entries, bf16).
    All four z quarters accumulate into ONE 2-bank PSUM tile, so PSUM
    staging is a single 1024-wide ACT copy (PSUM f32 -> SBUF bf16).
  Stage 2 (H4 butterfly over a, 256 cols/op, all-SBUF bf16): dist-2
    w0..w3 and dist-1 y0/y1 on DVE (16-bit fast mode), y2/y3 on Pool;
    one full-row store per tile on SP (the shared HWDGE generator costs
    ~625 ns per DMA, so DMA instruction count is kept minimal).
  Startup: all 16 loads queued on SP up front; W rides the ACT queue; a
  burst of dummy matmuls ramps the PE p-state during the first loads.
  The final tile keeps y2/y3 on DVE and splits its store to shorten the
  drain tail.
"""

import numpy as np
from contextlib import ExitStack

import concourse.tile as tile
from concourse import bacc, bass_utils, mybir

N_CORES = 8
B, S, D = 4, 4096, 1024
ROWS = B * S                 # 16384
SHARD = ROWS // N_CORES      # 2048
NT = SHARD // 128            # 16 tiles of 128 rows per core
F32 = mybir.dt.float32
BF16 = mybir.dt.bfloat16

_cache = {}

CFG = {
    "xin_bufs": 16,
    "out_bufs": 8,
    "zs_bufs": 6,
    "w_bufs": 6,
    "z_bufs": 2,
    "warmup": 10,
}


def _build_nc(cfg=None):
    cfg = {**CFG, **(cfg or {})}
    nc = bacc.Bacc("TRN2", target_bir_lowering=False, debug=False)
    # xt: per tile t, xt[t*128+b', g*128+n] = x[t*128+n, g*128+b'] (bf16)
    xt_d = nc.dram_tensor("xt", [SHARD, D], BF16, kind="ExternalInput").ap()
    w_d = nc.dram_tensor("w", [128, 512], BF16, kind="ExternalInput").ap()
    o_d = nc.dram_tensor("out", [SHARD, D], BF16, kind="ExternalOutput").ap()

    with tile.TileContext(nc) as tc, ExitStack() as ctx:
        const_pool = ctx.enter_context(tc.tile_pool(name="const", bufs=1))
        xin_pool = ctx.enter_context(tc.tile_pool(name="xin", bufs=cfg["xin_bufs"]))
        out_pool = ctx.enter_context(tc.tile_pool(name="outp", bufs=cfg["out_bufs"]))
        zs_pool = ctx.enter_context(tc.tile_pool(name="zs", bufs=cfg["zs_bufs"]))
        wb_pool = ctx.enter_context(tc.tile_pool(name="wb", bufs=cfg["w_bufs"]))
        ps_zp = ctx.enter_context(
            tc.tile_pool(name="ps_zp", bufs=cfg["z_bufs"], space="PSUM"))

        # All 16 xt loads queued on SP up front; W rides the ACT queue.
        xt_tiles = []
        for it in range(NT):
            xt_sb = xin_pool.tile([128, D], BF16, tag="xt")
            nc.sync.dma_start(xt_sb[:], xt_d[it * 128:(it + 1) * 128, :])
            xt_tiles.append(xt_sb)

        W_sb = const_pool.tile([128, 512], BF16, tag="W")
        nc.scalar.dma_start(W_sb[:], w_d[:])

        # PE p-state warmup: dummy matmuls on a zeroed tile while the first
        # loads are in flight (reusing the z0 PSUM pool).
        Zb_sb = const_pool.tile([128, 256], BF16, tag="Zb")
        nc.vector.memset(Zb_sb[:], 0.0)
        for _ in range(cfg["warmup"]):
            warm_ps = ps_zp.tile([128, 1024], F32, tag="z")
            nc.tensor.matmul(warm_ps[:, 0:256], lhsT=Zb_sb[:, 0:128], rhs=Zb_sb[:],
                             start=True, stop=True)

        for it in range(NT):
            xt_sb = xt_tiles[it]
            last = it == NT - 1
            # all four z quarters share one 2-bank PSUM tile (four
            # accumulation groups): staging to SBUF is ONE 1024-wide ACT copy
            zp = ps_zp.tile([128, 1024], F32, tag="z")
            zs = zs_pool.tile([128, 1024], BF16, tag="zs")
            for a in range(4):
                zps = zp[:, a * 256:(a + 1) * 256]
                for j in range(2):
                    g = 2 * a + j
                    nc.tensor.matmul(
                        zps,
                        lhsT=xt_sb[:, g * 128:(g + 1) * 128],
                        rhs=W_sb[:, j * 256:(j + 1) * 256],
                        start=(j == 0),
                        stop=(j == 1),
                    )
            nc.scalar.copy(zs[:], zp[:])
            zs01 = zs[:, 0:512]
            zs23 = zs[:, 512:1024]

            # H4 butterfly over the a axis, all-SBUF in bf16. The dist-2
            # pairs are contiguous in zs ([z0|z1] +- [z2|z3]), so the stage
            # is just TWO 512-wide DVE ops: wA = [w0|w1], wB = [w2|w3].
            w = wb_pool.tile([128, 1024], BF16, tag="w")
            nc.vector.tensor_add(w[:, 0:512], zs01, zs23)
            nc.vector.tensor_sub(w[:, 512:1024], zs01, zs23)

            ob = out_pool.tile([128, D], BF16, tag="ob")
            row = o_d[it * 128:(it + 1) * 128, :]
            # dist-1 stage, strided 3D views: both adds (y0 into ob[0:256],
            # y2 into ob[512:768]) are ONE DVE op over [2,256]; both subs are
            # ONE Pool op. One full store per tile on SP (~625 ns of HWDGE
            # per DMA). The final tile keeps the subs on DVE and splits its
            # store so the drain tail is as short as possible.
            wv = w[:].rearrange("p (h q c) -> p h q c", h=2, q=2)
            obv = ob[:].rearrange("p (h q c) -> p h q c", h=2, q=2)
            nc.vector.tensor_add(obv[:, :, 0, :], wv[:, :, 0, :], wv[:, :, 1, :])
            if last:
                nc.vector.tensor_sub(ob[:, 256:512], w[:, 0:256], w[:, 256:512])
                nc.sync.dma_start(row[:, 0:512], ob[:, 0:512])
                nc.vector.tensor_sub(ob[:, 768:1024], w[:, 512:768], w[:, 768:1024])
                nc.sync.dma_start(row[:, 512:1024], ob[:, 512:1024])
            else:
                nc.gpsimd.tensor_sub(obv[:, :, 1, :], wv[:, :, 0, :], wv[:, :, 1, :])
                nc.sync.dma_start(row[:], ob[:])

    nc.compile()
    return nc


def _get_nc():
    if "nc" not in _cache:
        _cache["nc"] = _build_nc()
    return _cache["nc"]


def kernel(x, H, **_ignored):
    import ml_dtypes

    x = np.asarray(x, dtype=np.float32)
    H = np.asarray(H, dtype=np.float32)
    nc = _get_nc()

    # Derive the Kronecker factors from the given H (exact when H has the
    # Hadamard structure); fold in the 1/sqrt(1024) scale.
    R = np.ascontiguousarray(H[:128, :128]) * np.float32(1.0 / 32.0)  # symmetric
    H2s = np.ascontiguousarray(H[:2, :2])  # (-1)^popcount(i&j) signs
    # W2[b', j*256 + e8*128 + e_lo] = H2s[e8, j] * R[b', e_lo]
    W = np.ascontiguousarray(
        np.einsum("ej,bl->bjel", H2s, R).reshape(128, 512)
    ).astype(ml_dtypes.bfloat16)

    # Round x to bf16 (the on-chip pipeline would do the same before the
    # 16-bit matmuls) and pre-transpose per 128-row tile:
    # xt[t, b', g, n] = x[t, n, g, b']
    xb = x.reshape(ROWS // 128, 128, 8, 128).astype(ml_dtypes.bfloat16)
    xt = np.ascontiguousarray(xb.transpose(0, 3, 2, 1)).reshape(ROWS, D)

    in_maps = []
    for c in range(N_CORES):
        in_maps.append({
            "xt": np.ascontiguousarray(xt[c * SHARD:(c + 1) * SHARD]),
            "w": W,
        })

    res = bass_utils.run_bass_kernel_spmd(nc, in_maps, core_ids=list(range(N_CORES)))
    y = np.empty((ROWS, D, 2), dtype=np.float32)
    for c in range(N_CORES):
        y[c * SHARD:(c + 1) * SHARD, :, 0] = res.results[c]["out"].astype(np.float32)
    y[:, :, 1] = 0.0
    return y.reshape(B, S, D, 2)


# revision 29
# speedup vs baseline: 1.0545x; 1.0350x over previous
"""Hadamard transform kernel for Trainium2 (8 NeuronCores, SPMD data-parallel).

Computes y = (x @ H^T) / sqrt(D), padded with a zero imaginary plane ->
[B, S, D, 2], for x [4, 4096, 1024] fp32 and H the 1024-point Hadamard
matrix (H[i,j] = (-1)^popcount(i&j), symmetric, Kronecker-structured).

Precision/layout choices (all inside kernel(), tolerance is 2e-2):
  - x is rounded to bf16 and pre-transposed per 128-row tile on the host
    during sharding (pure layout + the same rounding the on-chip pipeline
    would apply): halves load traffic and removes all PE transposes.
  - The device writes the real plane in bf16 (host upcasts to fp32 and
    interleaves the zero imaginary plane): halves store traffic.
  Measured end-to-end relative error ~3e-3.

Per-core traffic: 4 MiB in + 4 MiB out + 0.13 MiB weights (~23.7 us of
DMA at the 360 GB/s roofline); the four compute engines are balanced at
~1.2-1.3 us/tile each, right at the per-tile DMA budget.

Math (shard of 2048 rows, 16 row-tiles of 128):
  H_1024 = H_4 (x) H_256  under d = a*256 + b, f = c*256 + e, with
  H_256[e, j*128+b'] = H2[e8, j] * H128[e_lo, b'] (e = e8*128 + e_lo).
  Stage 1 (PE, bf16): per quarter a in 0..4, 2 accumulating matmuls
    z_a += xt[:, (2a+j)*128:...]^T @ W2[:, j*256:(j+1)*256], where
    W2[b', j*256 + e8*128 + e_lo] = H2[e8,j] * H128[e_lo,b'] / 32
    (host-precomputed, exact +-2^-5 entries, bf16).
    All four z quarters accumulate into ONE 2-bank PSUM tile, so PSUM
    staging is a single 1024-wide ACT copy (PSUM f32 -> SBUF bf16).
  Stage 2 (H4 butterfly over a, 256 cols/op, all-SBUF bf16): dist-2
    w0..w3 and dist-1 y0/y1 on DVE (16-bit fast mode), y2/y3 on Pool;
    one full-row store per tile on SP (the shared HWDGE generator costs
    ~625 ns per DMA, so DMA instruction count is kept minimal).
  Startup: all 16 loads queued on SP up front; W rides the ACT queue; a
  burst of dummy matmuls ramps the PE p-state during the first loads.
  The final tile keeps y2/y3 on DVE and splits its store to shorten the
  drain tail.
"""

import numpy as np
from contextlib import ExitStack

import concourse.tile as tile
from concourse import bacc, bass_utils, mybir

N_CORES = 8
B, S, D = 4, 4096, 1024
ROWS = B * S                 # 16384
SHARD = ROWS // N_CORES      # 2048
NT = SHARD // 128            # 16 tiles of 128 rows per core
F32 = mybir.dt.float32
BF16 = mybir.dt.bfloat16

_cache = {}

CFG = {
    "xin_bufs": 16,
    "out_bufs": 8,
    "zs_bufs": 6,
    "w_bufs": 6,
    "z_bufs": 3,
    "warmup": 10,
}


def _build_nc(cfg=None):
    cfg = {**CFG, **(cfg or {})}
    nc = bacc.Bacc("TRN2", target_bir_lowering=False, debug=False)
    # xt: per tile t, xt[t*128+b', g*128+n] = x[t*128+n, g*128+b'] (bf16)
    xt_d = nc.dram_tensor("xt", [SHARD, D], BF16, kind="ExternalInput").ap()
    w_d = nc.dram_tensor("w", [128, 512], BF16, kind="ExternalInput").ap()
    o_d = nc.dram_tensor("out", [SHARD, D], BF16, kind="ExternalOutput").ap()

    with tile.TileContext(nc) as tc, ExitStack() as ctx:
        const_pool = ctx.enter_context(tc.tile_pool(name="const", bufs=1))
        xin_pool = ctx.enter_context(tc.tile_pool(name="xin", bufs=cfg["xin_bufs"]))
        out_pool = ctx.enter_context(tc.tile_pool(name="outp", bufs=cfg["out_bufs"]))
        zs_pool = ctx.enter_context(tc.tile_pool(name="zs", bufs=cfg["zs_bufs"]))
        wb_pool = ctx.enter_context(tc.tile_pool(name="wb", bufs=cfg["w_bufs"]))
        ps_zp = ctx.enter_context(
            tc.tile_pool(name="ps_zp", bufs=cfg["z_bufs"], space="PSUM"))

        # All 16 xt loads queued on SP up front; W rides the ACT queue.
        xt_tiles = []
        for it in range(NT):
            xt_sb = xin_pool.tile([128, D], BF16, tag="xt")
            nc.sync.dma_start(xt_sb[:], xt_d[it * 128:(it + 1) * 128, :])
            xt_tiles.append(xt_sb)

        W_sb = const_pool.tile([128, 512], BF16, tag="W")
        nc.scalar.dma_start(W_sb[:], w_d[:])

        # PE p-state warmup: dummy matmuls on a zeroed tile while the first
        # loads are in flight (reusing the z0 PSUM pool).
        Zb_sb = const_pool.tile([128, 256], BF16, tag="Zb")
        nc.vector.memset(Zb_sb[:], 0.0)
        for _ in range(cfg["warmup"]):
            warm_ps = ps_zp.tile([128, 1024], F32, tag="z")
            nc.tensor.matmul(warm_ps[:, 0:256], lhsT=Zb_sb[:, 0:128], rhs=Zb_sb[:],
                             start=True, stop=True)

        for it in range(NT):
            xt_sb = xt_tiles[it]
            last = it == NT - 1
            # all four z quarters share one 2-bank PSUM tile (four
            # accumulation groups): staging to SBUF is ONE 1024-wide ACT copy
            zp = ps_zp.tile([128, 1024], F32, tag="z")
            zs = zs_pool.tile([128, 1024], BF16, tag="zs")
            for a in range(4):
                zps = zp[:, a * 256:(a + 1) * 256]
                for j in range(2):
                    g = 2 * a + j
                    nc.tensor.matmul(
                        zps,
                        lhsT=xt_sb[:, g * 128:(g + 1) * 128],
                        rhs=W_sb[:, j * 256:(j + 1) * 256],
                        start=(j == 0),
                        stop=(j == 1),
                    )
            nc.scalar.copy(zs[:], zp[:])
            zs01 = zs[:, 0:512]
            zs23 = zs[:, 512:1024]

            # H4 butterfly over the a axis, all-SBUF in bf16. The dist-2
            # pairs are contiguous in zs ([z0|z1] +- [z2|z3]), so the stage
            # is just TWO 512-wide DVE ops: wA = [w0|w1], wB = [w2|w3].
            w = wb_pool.tile([128, 1024], BF16, tag="w")
            nc.vector.tensor_add(w[:, 0:512], zs01, zs23)
            nc.vector.tensor_sub(w[:, 512:1024], zs01, zs23)

            ob = out_pool.tile([128, D], BF16, tag="ob")
            row = o_d[it * 128:(it + 1) * 128, :]
            # dist-1 stage, strided 3D views: both adds (y0 into ob[0:256],
            # y2 into ob[512:768]) are ONE DVE op over [2,256]; both subs are
            # ONE Pool op. One full store per tile on SP (~625 ns of HWDGE
            # per DMA). The final tile keeps the subs on DVE and splits its
            # store so the drain tail is as short as possible.
            wv = w[:].rearrange("p (h q c) -> p h q c", h=2, q=2)
            obv = ob[:].rearrange("p (h q c) -> p h q c", h=2, q=2)
            nc.vector.tensor_add(obv[:, :, 0, :], wv[:, :, 0, :], wv[:, :, 1, :])
            if last:
                nc.vector.tensor_sub(ob[:, 256:512], w[:, 0:256], w[:, 256:512])
                nc.sync.dma_start(row[:, 0:512], ob[:, 0:512])
                nc.vector.tensor_sub(ob[:, 768:1024], w[:, 512:768], w[:, 768:1024])
                nc.sync.dma_start(row[:, 512:1024], ob[:, 512:1024])
            else:
                nc.gpsimd.tensor_sub(obv[:, :, 1, :], wv[:, :, 0, :], wv[:, :, 1, :])
                nc.sync.dma_start(row[:], ob[:])

    nc.compile()
    return nc


def _get_nc():
    if "nc" not in _cache:
        _cache["nc"] = _build_nc()
    return _cache["nc"]


def kernel(x, H, **_ignored):
    import ml_dtypes

    x = np.asarray(x, dtype=np.float32)
    H = np.asarray(H, dtype=np.float32)
    nc = _get_nc()

    # Derive the Kronecker factors from the given H (exact when H has the
    # Hadamard structure); fold in the 1/sqrt(1024) scale.
    R = np.ascontiguousarray(H[:128, :128]) * np.float32(1.0 / 32.0)  # symmetric
    H2s = np.ascontiguousarray(H[:2, :2])  # (-1)^popcount(i&j) signs
    # W2[b', j*256 + e8*128 + e_lo] = H2s[e8, j] * R[b', e_lo]
    W = np.ascontiguousarray(
        np.einsum("ej,bl->bjel", H2s, R).reshape(128, 512)
    ).astype(ml_dtypes.bfloat16)

    # Round x to bf16 (the on-chip pipeline would do the same before the
    # 16-bit matmuls) and pre-transpose per 128-row tile:
    # xt[t, b', g, n] = x[t, n, g, b']
    xb = x.reshape(ROWS // 128, 128, 8, 128).astype(ml_dtypes.bfloat16)
    xt = np.ascontiguousarray(xb.transpose(0, 3, 2, 1)).reshape(ROWS, D)

    in_maps = []
    for c in range(N_CORES):
        in_maps.append({
            "xt": np.ascontiguousarray(xt[c * SHARD:(c + 1) * SHARD]),
            "w": W,
        })

    res = bass_utils.run_bass_kernel_spmd(nc, in_maps, core_ids=list(range(N_CORES)))
    y = np.empty((ROWS, D, 2), dtype=np.float32)
    for c in range(N_CORES):
        y[c * SHARD:(c + 1) * SHARD, :, 0] = res.results[c]["out"].astype(np.float32)
    y[:, :, 1] = 0.0
    return y.reshape(B, S, D, 2)


# revision 42
# speedup vs baseline: 1.0718x; 1.0164x over previous
"""Hadamard transform kernel for Trainium2 (8 NeuronCores, SPMD data-parallel).

Computes y = (x @ H^T) / sqrt(D), padded with a zero imaginary plane ->
[B, S, D, 2], for x [4, 4096, 1024] fp32 and H the 1024-point Hadamard
matrix (H[i,j] = (-1)^popcount(i&j), symmetric, Kronecker-structured).

Precision/layout choices (all inside kernel(), tolerance is 2e-2):
  - x is rounded to bf16 and pre-transposed per 128-row tile on the host
    during sharding (pure layout + the same rounding the on-chip pipeline
    would apply): halves load traffic and removes all PE transposes.
  - The device writes the real plane in bf16 (host upcasts to fp32 and
    interleaves the zero imaginary plane): halves store traffic.
  Measured end-to-end relative error ~3e-3.

Per-core traffic: 4 MiB in + 4 MiB out + 0.13 MiB weights (~23.7 us of
DMA at the 360 GB/s roofline); the four compute engines are balanced at
~1.2-1.3 us/tile each, right at the per-tile DMA budget.

Math (shard of 2048 rows, 16 row-tiles of 128):
  H_1024 = H_4 (x) H_256  under d = a*256 + b, f = c*256 + e, with
  H_256[e, j*128+b'] = H2[e8, j] * H128[e_lo, b'] (e = e8*128 + e_lo).
  Stage 1 (PE, bf16): per quarter a in 0..4, 2 accumulating matmuls
    z_a += xt[:, (2a+j)*128:...]^T @ W2[:, j*256:(j+1)*256], where
    W2[b', j*256 + e8*128 + e_lo] = H2[e8,j] * H128[e_lo,b'] / 32
    (host-precomputed, exact +-2^-5 entries, bf16).
    All four z quarters accumulate into ONE 2-bank PSUM tile, so PSUM
    staging is a single 1024-wide ACT copy (PSUM f32 -> SBUF bf16).
  Stage 2 (H4 butterfly over a, 256 cols/op, all-SBUF bf16): dist-2
    w0..w3 and dist-1 y0/y1 on DVE (16-bit fast mode), y2/y3 on Pool;
    one full-row store per tile on SP (the shared HWDGE generator costs
    ~625 ns per DMA, so DMA instruction count is kept minimal).
  Startup: all 16 loads queued on SP up front; W rides the ACT queue; a
  burst of dummy matmuls ramps the PE p-state during the first loads.
  The final tile keeps y2/y3 on DVE and splits its store to shorten the
  drain tail.
"""

import numpy as np
from contextlib import ExitStack

import concourse.tile as tile
from concourse import bacc, bass_utils, mybir

N_CORES = 8
B, S, D = 4, 4096, 1024
ROWS = B * S                 # 16384
SHARD = ROWS // N_CORES      # 2048
NT = SHARD // 128            # 16 tiles of 128 rows per core
F32 = mybir.dt.float32
BF16 = mybir.dt.bfloat16

_cache = {}

CFG = {
    "xin_bufs": 16,
    "out_bufs": 8,
    "zs_bufs": 10,
    "w_bufs": 10,
    "z_bufs": 3,
    "warmup": 10,
    "sub_split": 240,
}


def _build_nc(cfg=None):
    cfg = {**CFG, **(cfg or {})}
    nc = bacc.Bacc("TRN2", target_bir_lowering=False, debug=False)
    # xt: per tile t, xt[t*128+b', g*128+n] = x[t*128+n, g*128+b'] (bf16)
    xt_d = nc.dram_tensor("xt", [SHARD, D], BF16, kind="ExternalInput").ap()
    w_d = nc.dram_tensor("w", [128, 512], BF16, kind="ExternalInput").ap()
    o_d = nc.dram_tensor("out", [SHARD, D], BF16, kind="ExternalOutput").ap()

    with tile.TileContext(nc) as tc, ExitStack() as ctx:
        const_pool = ctx.enter_context(tc.tile_pool(name="const", bufs=1))
        xin_pool = ctx.enter_context(tc.tile_pool(name="xin", bufs=cfg["xin_bufs"]))
        out_pool = ctx.enter_context(tc.tile_pool(name="outp", bufs=cfg["out_bufs"]))
        zs_pool = ctx.enter_context(tc.tile_pool(name="zs", bufs=cfg["zs_bufs"]))
        wb_pool = ctx.enter_context(tc.tile_pool(name="wb", bufs=cfg["w_bufs"]))
        ps_zp = ctx.enter_context(
            tc.tile_pool(name="ps_zp", bufs=cfg["z_bufs"], space="PSUM"))

        # All 16 xt loads queued on SP up front; W rides the ACT queue.
        xt_tiles = []
        W_sb = const_pool.tile([128, 512], BF16, tag="W")
        for it in range(NT):
            xt_sb = xin_pool.tile([128, D], BF16, tag="xt")
            nc.sync.dma_start(xt_sb[:], xt_d[it * 128:(it + 1) * 128, :])
            xt_tiles.append(xt_sb)
            if it == 0:
                # W rides SP right after x0: the load stream stays gapless
                # and W lands before the first matmul needs it
                nc.sync.dma_start(W_sb[:], w_d[:])

        # PE p-state warmup: dummy matmuls on a zeroed tile while the first
        # loads are in flight (reusing the z0 PSUM pool).
        Zb_sb = const_pool.tile([128, 256], BF16, tag="Zb")
        nc.vector.memset(Zb_sb[:], 0.0)
        for _ in range(cfg["warmup"]):
            warm_ps = ps_zp.tile([128, 1024], F32, tag="z")
            nc.tensor.matmul(warm_ps[:, 0:256], lhsT=Zb_sb[:, 0:128], rhs=Zb_sb[:],
                             start=True, stop=True)

        for it in range(NT):
            xt_sb = xt_tiles[it]
            last = it == NT - 1
            # all four z quarters share one 2-bank PSUM tile (four
            # accumulation groups): staging to SBUF is ONE 1024-wide ACT copy
            zp = ps_zp.tile([128, 1024], F32, tag="z")
            zs = zs_pool.tile([128, 1024], BF16, tag="zs")
            for a in range(4):
                zps = zp[:, a * 256:(a + 1) * 256]
                for j in range(2):
                    g = 2 * a + j
                    nc.tensor.matmul(
                        zps,
                        lhsT=xt_sb[:, g * 128:(g + 1) * 128],
                        rhs=W_sb[:, j * 256:(j + 1) * 256],
                        start=(j == 0),
                        stop=(j == 1),
                    )
            nc.scalar.copy(zs[:], zp[:])
            zs01 = zs[:, 0:512]
            zs23 = zs[:, 512:1024]

            # H4 butterfly over the a axis, all-SBUF in bf16. The dist-2
            # pairs are contiguous in zs ([z0|z1] +- [z2|z3]), so the stage
            # is just TWO 512-wide DVE ops: wA = [w0|w1], wB = [w2|w3].
            w = wb_pool.tile([128, 1024], BF16, tag="w")
            nc.vector.tensor_add(w[:, 0:512], zs01, zs23)
            nc.vector.tensor_sub(w[:, 512:1024], zs01, zs23)

            ob = out_pool.tile([128, D], BF16, tag="ob")
            row = o_d[it * 128:(it + 1) * 128, :]
            # dist-1 stage, strided 3D views: both adds (y0 into ob[0:256],
            # y2 into ob[512:768]) are ONE DVE op over [2,256]; both subs are
            # ONE Pool op. One full store per tile on SP (~625 ns of HWDGE
            # per DMA). The final tile keeps the subs on DVE and splits its
            # store so the drain tail is as short as possible.
            wv = w[:].rearrange("p (h q c) -> p h q c", h=2, q=2)
            obv = ob[:].rearrange("p (h q c) -> p h q c", h=2, q=2)
            nc.vector.tensor_add(obv[:, :, 0, :], wv[:, :, 0, :], wv[:, :, 1, :])
            if last:
                nc.vector.tensor_sub(ob[:, 256:512], w[:, 0:256], w[:, 256:512])
                nc.sync.dma_start(row[:, 0:512], ob[:, 0:512])
                nc.vector.tensor_sub(ob[:, 768:1024], w[:, 512:768], w[:, 768:1024])
                nc.sync.dma_start(row[:, 512:1024], ob[:, 512:1024])
            else:
                # split the sub columns Pool/DVE so both stay at ~ACT's level
                c0 = cfg["sub_split"]
                nc.gpsimd.tensor_sub(obv[:, :, 1, 0:c0],
                                     wv[:, :, 0, 0:c0], wv[:, :, 1, 0:c0])
                nc.vector.tensor_sub(obv[:, :, 1, c0:256],
                                     wv[:, :, 0, c0:256], wv[:, :, 1, c0:256])
                nc.sync.dma_start(row[:], ob[:])

    nc.compile()
    return nc


def _get_nc():
    if "nc" not in _cache:
        _cache["nc"] = _build_nc()
    return _cache["nc"]


def kernel(x, H, **_ignored):
    import ml_dtypes

    x = np.asarray(x, dtype=np.float32)
    H = np.asarray(H, dtype=np.float32)
    nc = _get_nc()

    # Derive the Kronecker factors from the given H (exact when H has the
    # Hadamard structure); fold in the 1/sqrt(1024) scale.
    R = np.ascontiguousarray(H[:128, :128]) * np.float32(1.0 / 32.0)  # symmetric
    H2s = np.ascontiguousarray(H[:2, :2])  # (-1)^popcount(i&j) signs
    # W2[b', j*256 + e8*128 + e_lo] = H2s[e8, j] * R[b', e_lo]
    W = np.ascontiguousarray(
        np.einsum("ej,bl->bjel", H2s, R).reshape(128, 512)
    ).astype(ml_dtypes.bfloat16)

    # Round x to bf16 (the on-chip pipeline would do the same before the
    # 16-bit matmuls) and pre-transpose per 128-row tile:
    # xt[t, b', g, n] = x[t, n, g, b']
    xb = x.reshape(ROWS // 128, 128, 8, 128).astype(ml_dtypes.bfloat16)
    xt = np.ascontiguousarray(xb.transpose(0, 3, 2, 1)).reshape(ROWS, D)

    in_maps = []
    for c in range(N_CORES):
        in_maps.append({
            "xt": np.ascontiguousarray(xt[c * SHARD:(c + 1) * SHARD]),
            "w": W,
        })

    res = bass_utils.run_bass_kernel_spmd(nc, in_maps, core_ids=list(range(N_CORES)))
    y = np.empty((ROWS, D, 2), dtype=np.float32)
    for c in range(N_CORES):
        y[c * SHARD:(c + 1) * SHARD, :, 0] = res.results[c]["out"].astype(np.float32)
    y[:, :, 1] = 0.0
    return y.reshape(B, S, D, 2)


# revision 43
# speedup vs baseline: 1.0773x; 1.0051x over previous
"""Hadamard transform kernel for Trainium2 (8 NeuronCores, SPMD data-parallel).

Computes y = (x @ H^T) / sqrt(D), padded with a zero imaginary plane ->
[B, S, D, 2], for x [4, 4096, 1024] fp32 and H the 1024-point Hadamard
matrix (H[i,j] = (-1)^popcount(i&j), symmetric, Kronecker-structured).

Precision/layout choices (all inside kernel(), tolerance is 2e-2):
  - x is rounded to bf16 and pre-transposed per 128-row tile on the host
    during sharding (pure layout + the same rounding the on-chip pipeline
    would apply): halves load traffic and removes all PE transposes.
  - The device writes the real plane in bf16 (host upcasts to fp32 and
    interleaves the zero imaginary plane): halves store traffic.
  Measured end-to-end relative error ~3e-3.

Per-core traffic: 4 MiB in + 4 MiB out + 0.13 MiB weights (~23.7 us of
DMA at the 360 GB/s roofline); the four compute engines are balanced at
~1.2-1.3 us/tile each, right at the per-tile DMA budget.

Math (shard of 2048 rows, 16 row-tiles of 128):
  H_1024 = H_4 (x) H_256  under d = a*256 + b, f = c*256 + e, with
  H_256[e, j*128+b'] = H2[e8, j] * H128[e_lo, b'] (e = e8*128 + e_lo).
  Stage 1 (PE, bf16): per quarter a in 0..4, 2 accumulating matmuls
    z_a += xt[:, (2a+j)*128:...]^T @ W2[:, j*256:(j+1)*256], where
    W2[b', j*256 + e8*128 + e_lo] = H2[e8,j] * H128[e_lo,b'] / 32
    (host-precomputed, exact +-2^-5 entries, bf16).
    All four z quarters accumulate into ONE 2-bank PSUM tile, so PSUM
    staging is a single 1024-wide ACT copy (PSUM f32 -> SBUF bf16).
  Stage 2 (H4 butterfly over a, 256 cols/op, all-SBUF bf16): dist-2
    w0..w3 and dist-1 y0/y1 on DVE (16-bit fast mode), y2/y3 on Pool;
    one full-row store per tile on SP (the shared HWDGE generator costs
    ~625 ns per DMA, so DMA instruction count is kept minimal).
  Startup: all 16 loads queued on SP up front; W rides the ACT queue; a
  burst of dummy matmuls ramps the PE p-state during the first loads.
  The final tile keeps y2/y3 on DVE and splits its store to shorten the
  drain tail.
"""

import numpy as np
from contextlib import ExitStack

import concourse.tile as tile
from concourse import bacc, bass_utils, mybir

N_CORES = 8
B, S, D = 4, 4096, 1024
ROWS = B * S                 # 16384
SHARD = ROWS // N_CORES      # 2048
NT = SHARD // 128            # 16 tiles of 128 rows per core
F32 = mybir.dt.float32
BF16 = mybir.dt.bfloat16

_cache = {}

CFG = {
    "xin_bufs": 16,
    "out_bufs": 8,
    "zs_bufs": 10,
    "w_bufs": 10,
    "z_bufs": 3,
    "warmup": 10,
    "sub_split": 240,
}


def _build_nc(cfg=None):
    cfg = {**CFG, **(cfg or {})}
    nc = bacc.Bacc("TRN2", target_bir_lowering=False, debug=False)
    # xt: per tile t, xt[t*128+b', g*128+n] = x[t*128+n, g*128+b'] (bf16)
    xt_d = nc.dram_tensor("xt", [SHARD, D], BF16, kind="ExternalInput").ap()
    w_d = nc.dram_tensor("w", [128, 512], BF16, kind="ExternalInput").ap()
    o_d = nc.dram_tensor("out", [SHARD, D], BF16, kind="ExternalOutput").ap()

    with tile.TileContext(nc) as tc, ExitStack() as ctx:
        const_pool = ctx.enter_context(tc.tile_pool(name="const", bufs=1))
        xin_pool = ctx.enter_context(tc.tile_pool(name="xin", bufs=cfg["xin_bufs"]))
        out_pool = ctx.enter_context(tc.tile_pool(name="outp", bufs=cfg["out_bufs"]))
        zs_pool = ctx.enter_context(tc.tile_pool(name="zs", bufs=cfg["zs_bufs"]))
        wb_pool = ctx.enter_context(tc.tile_pool(name="wb", bufs=cfg["w_bufs"]))
        ps_zp = ctx.enter_context(
            tc.tile_pool(name="ps_zp", bufs=cfg["z_bufs"], space="PSUM"))

        # All 16 xt loads queued on SP up front; W rides the ACT queue.
        xt_tiles = []
        W_sb = const_pool.tile([128, 512], BF16, tag="W")
        for it in range(NT):
            xt_sb = xin_pool.tile([128, D], BF16, tag="xt")
            nc.sync.dma_start(xt_sb[:], xt_d[it * 128:(it + 1) * 128, :])
            xt_tiles.append(xt_sb)
            if it == 0:
                # W rides SP right after x0: the load stream stays gapless
                # and W lands before the first matmul needs it
                nc.sync.dma_start(W_sb[:], w_d[:])

        # PE p-state warmup: dummy matmuls on a zeroed tile while the first
        # loads are in flight (reusing the z0 PSUM pool).
        Zb_sb = const_pool.tile([128, 256], BF16, tag="Zb")
        nc.vector.memset(Zb_sb[:], 0.0)
        for _ in range(cfg["warmup"]):
            warm_ps = ps_zp.tile([128, 1024], F32, tag="z")
            nc.tensor.matmul(warm_ps[:, 0:256], lhsT=Zb_sb[:, 0:128], rhs=Zb_sb[:],
                             start=True, stop=True)

        for it in range(NT):
            xt_sb = xt_tiles[it]
            last = it == NT - 1
            # all four z quarters share one 2-bank PSUM tile (four
            # accumulation groups): staging to SBUF is ONE 1024-wide ACT copy
            zp = ps_zp.tile([128, 1024], F32, tag="z")
            zs = zs_pool.tile([128, 1024], BF16, tag="zs")
            for a in range(4):
                zps = zp[:, a * 256:(a + 1) * 256]
                for j in range(2):
                    g = 2 * a + j
                    nc.tensor.matmul(
                        zps,
                        lhsT=xt_sb[:, g * 128:(g + 1) * 128],
                        rhs=W_sb[:, j * 256:(j + 1) * 256],
                        start=(j == 0),
                        stop=(j == 1),
                    )
            nc.scalar.copy(zs[:], zp[:])
            zs01 = zs[:, 0:512]
            zs23 = zs[:, 512:1024]

            # H4 butterfly over the a axis, all-SBUF in bf16. The dist-2
            # pairs are contiguous in zs ([z0|z1] +- [z2|z3]), so the stage
            # is just TWO 512-wide DVE ops: wA = [w0|w1], wB = [w2|w3].
            w = wb_pool.tile([128, 1024], BF16, tag="w")
            nc.vector.tensor_add(w[:, 0:512], zs01, zs23)
            nc.vector.tensor_sub(w[:, 512:1024], zs01, zs23)

            ob = out_pool.tile([128, D], BF16, tag="ob")
            row = o_d[it * 128:(it + 1) * 128, :]
            # dist-1 stage, strided 3D views: both adds (y0 into ob[0:256],
            # y2 into ob[512:768]) are ONE DVE op over [2,256]; both subs are
            # ONE Pool op. One full store per tile on SP (~625 ns of HWDGE
            # per DMA). The final tile keeps the subs on DVE and splits its
            # store so the drain tail is as short as possible.
            wv = w[:].rearrange("p (h q c) -> p h q c", h=2, q=2)
            obv = ob[:].rearrange("p (h q c) -> p h q c", h=2, q=2)
            nc.vector.tensor_add(obv[:, :, 0, :], wv[:, :, 0, :], wv[:, :, 1, :])
            if last:
                nc.vector.tensor_sub(ob[:, 256:512], w[:, 0:256], w[:, 256:512])
                nc.sync.dma_start(row[:, 0:512], ob[:, 0:512])
                nc.vector.tensor_sub(ob[:, 768:1024], w[:, 512:768], w[:, 768:1024])
                # the final store rides ACT (its queue is empty by now), so
                # its descriptor-gen runs parallel to SP's lo-half gen
                nc.scalar.dma_start(row[:, 512:1024], ob[:, 512:1024])
            else:
                # split the sub columns Pool/DVE so both stay at ~ACT's level
                c0 = cfg["sub_split"]
                nc.gpsimd.tensor_sub(obv[:, :, 1, 0:c0],
                                     wv[:, :, 0, 0:c0], wv[:, :, 1, 0:c0])
                nc.vector.tensor_sub(obv[:, :, 1, c0:256],
                                     wv[:, :, 0, c0:256], wv[:, :, 1, c0:256])
                nc.sync.dma_start(row[:], ob[:])

    nc.compile()
    return nc


def _get_nc():
    if "nc" not in _cache:
        _cache["nc"] = _build_nc()
    return _cache["nc"]


def kernel(x, H, **_ignored):
    import ml_dtypes

    x = np.asarray(x, dtype=np.float32)
    H = np.asarray(H, dtype=np.float32)
    nc = _get_nc()

    # Derive the Kronecker factors from the given H (exact when H has the
    # Hadamard structure); fold in the 1/sqrt(1024) scale.
    R = np.ascontiguousarray(H[:128, :128]) * np.float32(1.0 / 32.0)  # symmetric
    H2s = np.ascontiguousarray(H[:2, :2])  # (-1)^popcount(i&j) signs
    # W2[b', j*256 + e8*128 + e_lo] = H2s[e8, j] * R[b', e_lo]
    W = np.ascontiguousarray(
        np.einsum("ej,bl->bjel", H2s, R).reshape(128, 512)
    ).astype(ml_dtypes.bfloat16)

    # Round x to bf16 (the on-chip pipeline would do the same before the
    # 16-bit matmuls) and pre-transpose per 128-row tile:
    # xt[t, b', g, n] = x[t, n, g, b']
    xb = x.reshape(ROWS // 128, 128, 8, 128).astype(ml_dtypes.bfloat16)
    xt = np.ascontiguousarray(xb.transpose(0, 3, 2, 1)).reshape(ROWS, D)

    in_maps = []
    for c in range(N_CORES):
        in_maps.append({
            "xt": np.ascontiguousarray(xt[c * SHARD:(c + 1) * SHARD]),
            "w": W,
        })

    res = bass_utils.run_bass_kernel_spmd(nc, in_maps, core_ids=list(range(N_CORES)))
    y = np.empty((ROWS, D, 2), dtype=np.float32)
    for c in range(N_CORES):
        y[c * SHARD:(c + 1) * SHARD, :, 0] = res.results[c]["out"].astype(np.float32)
    y[:, :, 1] = 0.0
    return y.reshape(B, S, D, 2)
